# revision 1
# baseline (speedup 1.0000x reference)
"""BitSelfAttention on 8 TRN2 NeuronCores.

Sharding: core c handles batch b = c//2 and head-group hg = c%2 (8 of 16 heads).
Each core computes its 8 heads' QKV projections + causal attention + its slice
of the o_proj GEMM, producing a partial output (transposed, [D, T], fp32).
Host: pre-quantizes BitLinear weights (ternary * gamma, exact in bf16),
pre-transposes operands into matmul-friendly layouts, and sums the two
head-group partials per batch at the end.

Device layouts (per core):
  xT   [D, T]  bf16 : x[b].T              (rhs for Q/K/V^T projections)
  wqT  [D, F]  bf16 : w_q_eff[hg-rows].T  (stationary tiles for Q^T proj)
  wkT  [D, F]  bf16
  wvT  [D, F]  bf16
  woT  [F, D]  bf16 : w_o_eff[:, hg-cols].T (stationary tiles for o_proj)
  cmask[4, 128, 512] bf16 : causal masks for the 4 diagonal offsets
  outT [D, T]  fp32 : partial output, transposed

Per head h: Q^T,K^T [dh=128, T] (dh-major), V^T transposed on the PE into
token-major V tiles. Attention computed as S^T = K^T_tile.T @ Q^T_block so
softmax rows land on the free axis; P^T = exp(S^T*scale) (ACT, PSUM->SBUF
bf16); key-tile partial row-sums accumulate in fp32 on the vector engine and
one all-ones stationary matmul per block reduces across partitions while
broadcasting the result to every partition (so normalization needs no
cross-partition broadcast); O^T = V_tile.T @ P^T accumulated over key tiles;
normalize with fast-reciprocal+multiply during PSUM eviction. o_proj consumes
O^T tiles directly as stationary operands, producing outT; its per-token-block
chains double as PE fill work zipped into the last head's attention, just as
each head's projection chains are zipped into the previous head's attention
(the attention inner loop is otherwise exp-latency-gated on the in-order PE).
"""

import math

import ml_dtypes
import numpy as np

import concourse.mybir as mybir
import concourse.tile as tile
from concourse import bacc
from concourse import bass_utils
from concourse.masks import make_identity

BF16 = mybir.dt.bfloat16
F32 = mybir.dt.float32

D_MODEL = 2048
N_HEAD = 16
D_HEAD = 128
B = 4
T_FULL = 2048
N_CORES = 8
F_LOC = D_MODEL // 2  # features per core (8 heads)


def build_bass(T=T_FULL, D=D_MODEL, F=F_LOC, debug=False):
    """Build the single-core program (SPMD across 8 cores via input data)."""
    P = 128
    KD = D // P      # contraction 128-tiles
    TT = T // P      # token 128-tiles
    TB = T // 512    # token 512-blocks
    H = F // P       # local heads
    MT = D // P      # output-dmodel 128-tiles
    KT_PER_B = 512 // P
    SCALE = 1.0 / math.sqrt(D_HEAD)

    nc = bacc.Bacc("TRN2", target_bir_lowering=False, debug=debug,
                   num_devices=N_CORES)
    xT_d = nc.dram_tensor("xT", [D, T], BF16, kind="ExternalInput").ap()
    # weights pre-tiled on host into the exact SBUF layouts (contiguous DMAs):
    #   wqT/wkT/wvT: [H, 128, KD*128] with [h, p, kd*128+f] = w_eff[h*128+f, kd*128+p]
    #   woT:         [MT, 128, H*128] with [m, p, h*128+j] = wo_eff[m*128+j, h*128+p]
    H_ = F // P
    MT_ = D // P
    KD_ = D // P
    wqT_d = nc.dram_tensor("wqT", [H_, P, KD_ * P], BF16,
                           kind="ExternalInput").ap()
    wkT_d = nc.dram_tensor("wkT", [H_, P, KD_ * P], BF16,
                           kind="ExternalInput").ap()
    wvT_d = nc.dram_tensor("wvT", [H_, P, KD_ * P], BF16,
                           kind="ExternalInput").ap()
    woT_d = nc.dram_tensor("woT", [MT_, P, H_ * P], BF16,
                           kind="ExternalInput").ap()
    cm_d = nc.dram_tensor("cmask", [4, P, 512], BF16, kind="ExternalInput").ap()
    out_d = nc.dram_tensor("outT", [D, T], F32, kind="ExternalOutput").ap()

    with tile.TileContext(nc) as tc:
        with (
            tc.tile_pool(name="big", bufs=1) as big,
            tc.tile_pool(name="work", bufs=2) as work,
            tc.tile_pool(name="psS", bufs=3, space="PSUM") as psS,
            tc.tile_pool(name="psO", bufs=2, space="PSUM") as psO,
            tc.tile_pool(name="psR", bufs=1, space="PSUM") as psR,
            tc.tile_pool(name="psP", bufs=2, space="PSUM") as psP,
        ):
            # ---- persistent inputs (head-0 weights first: first MMs need them)
            wvh0 = work.tile([P, KD, P], BF16, name="wvh0", tag="wvh")
            nc.sync.dma_start(out=wvh0.rearrange("p kd f -> p (kd f)"),
                              in_=wvT_d[0])
            ones = big.tile([P, P], BF16, name="ones_sb", tag="ones", bufs=1)
            nc.vector.memset(ones, 1.0)
            ident = big.tile([P, P], BF16, name="ident_sb", tag="ident", bufs=1)
            make_identity(nc, ident)
            xt = []
            for kd in range(KD):
                xti = big.tile([P, T], BF16, name=f"xt{kd}", tag="xt", bufs=KD)
                nc.sync.dma_start(out=xti, in_=xT_d[kd * P:(kd + 1) * P, :])
                xt.append(xti)
            cmask = big.tile([P, 4, 512], BF16, name="cmask_sb", tag="cmask",
                             bufs=1)
            for i in range(4):
                nc.sync.dma_start(out=cmask[:, i, :], in_=cm_d[i])
            ot = [big.tile([P, T], BF16, name=f"ot{h}", tag="ot", bufs=H)
                  for h in range(H)]

            # ---- per-head pipeline with cross-head fill interleaving.
            # The attention inner loop is ACT(exp)-gated by ~40ns/iter; we
            # pump one projection matmul of the NEXT head between attention
            # iterations so the (in-order) PE always has fill work.
            def load_head_weights(h, wvh=None):
                if wvh is None:
                    wvh = work.tile([P, KD, P], BF16, name=f"wvh{h}",
                                    tag="wvh")
                    nc.sync.dma_start(out=wvh.rearrange("p kd f -> p (kd f)"),
                                      in_=wvT_d[h])
                wqh = work.tile([P, KD, P], BF16, name=f"wqh{h}", tag="wqh")
                nc.sync.dma_start(out=wqh.rearrange("p kd f -> p (kd f)"),
                                  in_=wqT_d[h])
                wkh = work.tile([P, KD, P], BF16, name=f"wkh{h}", tag="wkh")
                nc.sync.dma_start(out=wkh.rearrange("p kd f -> p (kd f)"),
                                  in_=wkT_d[h])
                return wqh, wkh, wvh

            def load_qk_weights(h):
                wqh = work.tile([P, KD, P], BF16, name=f"wqh{h}", tag="wqh")
                nc.sync.dma_start(out=wqh.rearrange("p kd f -> p (kd f)"),
                                  in_=wqT_d[h])
                wkh = work.tile([P, KD, P], BF16, name=f"wkh{h}", tag="wkh")
                nc.sync.dma_start(out=wkh.rearrange("p kd f -> p (kd f)"),
                                  in_=wkT_d[h])
                return wqh, wkh

            def alloc_head_tiles(h):
                vT = work.tile([P, T], BF16, name=f"vT{h}", tag="vT")
                vh = work.tile([P, TT, P], BF16, name=f"vh{h}", tag="vh")
                qt_ = work.tile([P, T], BF16, name=f"qt{h}", tag="qt")
                kt_ = work.tile([P, T], BF16, name=f"kt{h}", tag="kt")
                return vT, vh, qt_, kt_

            def proj_fill_gen(ws, tiles):
                """V^T then Q^T then K^T projection chains, yielding after
                every matmul so the caller can interleave them."""
                wqh, wkh, wvh = ws
                vT, vh, qt_, kt_ = tiles
                for wh, dst in ((wvh, vT), (wqh, qt_), (wkh, kt_)):
                    for tb in range(TB):
                        ts_ = slice(tb * 512, (tb + 1) * 512)
                        ps = psP.tile([P, 512], F32, name="psfill", tag="psp")
                        for kd in range(KD):
                            nc.tensor.matmul(ps, lhsT=wh[:, kd, :],
                                             rhs=xt[kd][:, ts_],
                                             start=(kd == 0),
                                             stop=(kd == KD - 1))
                            yield
                        nc.vector.tensor_copy(out=dst[:, ts_], in_=ps)

            def pump(gen, n):
                for _ in range(n):
                    try:
                        next(gen)
                    except StopIteration:
                        return False
                return True

            def pump_n(gen, n):
                c = 0
                for _ in range(n):
                    try:
                        next(gen)
                        c += 1
                    except StopIteration:
                        break
                return c

            def oproj_nb_gen(nb):
                """o_proj chains for one token block (needs all heads' ot
                columns of that block only), yielding per matmul."""
                ns = slice(nb * 512, (nb + 1) * 512)
                for m in range(MT):
                    woh = work.tile([P, H, P], BF16, name=f"woh{nb}_{m}",
                                    tag="woh", bufs=4)
                    nc.sync.dma_start(out=woh.rearrange("p h f -> p (h f)"),
                                      in_=woT_d[m])
                    yield  # let attention matmuls cover the woh DMA latency
                    ps = psP.tile([P, 512], F32, name="psout", tag="psp")
                    for hh in range(H):
                        nc.tensor.matmul(ps, lhsT=woh[:, hh, :],
                                         rhs=ot[hh][:, ns],
                                         start=(hh == 0), stop=(hh == H - 1))
                        yield
                    stg = work.tile([P, 512], F32, name="ostage", tag="ostage",
                                    bufs=4)
                    nc.vector.tensor_copy(out=stg, in_=ps)
                    nc.sync.dma_start(out=out_d[m * P:(m + 1) * P, ns],
                                      in_=stg)

            # head-0 Q/K weights and head-1 weights load after xt (the V^T
            # chains consume xt first; the Q chains run ~4 chain-times later)
            ws_list = [None] * (H + 2)
            wqh0, wkh0 = load_qk_weights(0)
            ws_list[0] = (wqh0, wkh0, wvh0)
            if H > 1:
                ws_list[1] = load_head_weights(1)
            cur_tiles = alloc_head_tiles(0)
            g0 = proj_fill_gen(ws_list[0], cur_tiles)
            while pump(g0, 1):
                pass

            fills = []

            def pump_fills(n):
                while n > 0 and fills:
                    n -= pump_n(fills[0], n)
                    if n > 0:
                        fills.pop(0)

            for h in range(H):
                vT, vh, qt_, kt_ = cur_tiles
                # prefetch weights two heads ahead so fill matmuls never
                # wait on their DMA (a blocked fill stalls the in-order PE)
                if h + 2 < H:
                    ws_list[h + 2] = load_head_weights(h + 2)
                if h + 1 < H:
                    next_tiles = alloc_head_tiles(h + 1)
                    fills.append(proj_fill_gen(ws_list[h + 1], next_tiles))
                else:
                    next_tiles = None

                def emit_transpose(kt):
                    # lives in the psS pool: psP slots are held long by
                    # in-flight interleaved fill chains
                    pst = psS.tile([P, 512], BF16, name="pst", tag="pss")
                    nc.tensor.transpose(pst[:, 0:P],
                                        vT[:, kt * P:(kt + 1) * P], ident)
                    nc.vector.tensor_copy(out=vh[:, kt, :], in_=pst[:, 0:P])

                # causal attention, S^T layout (keys on partitions).
                # Diagonal tiles (kt = 4*qb+di) only contribute to query
                # columns >= 128*di of the block; narrow S/exp/O/R to the
                # live columns. Only the first 128 columns of a (narrowed)
                # diagonal tile are triangular; the rest are fully allowed.
                for qb in range(TB):
                    nkt = KT_PER_B * (qb + 1)
                    for kt in range(KT_PER_B * qb, nkt):
                        emit_transpose(kt)
                    psO_t = psO.tile([P, 512], F32, name="psodt", tag="pso")
                    racc = work.tile([P, 512], F32, name="racc", tag="racc")
                    for kt in range(nkt):
                        di = kt - KT_PER_B * qb
                        c0 = max(di, 0) * P  # first live query column
                        w = 512 - c0
                        qs = slice(qb * 512 + c0, (qb + 1) * 512)
                        psS_t = psS.tile([P, 512], F32, name="pssc", tag="pss")
                        nc.tensor.matmul(psS_t[:, :w],
                                         lhsT=kt_[:, kt * P:(kt + 1) * P],
                                         rhs=qt_[:, qs],
                                         start=True, stop=True)
                        pt = work.tile([P, 512], BF16, name="pexp", tag="pt",
                                       bufs=6)
                        nc.scalar.activation(
                            out=pt[:, :w], in_=psS_t[:, :w],
                            func=mybir.ActivationFunctionType.Exp, scale=SCALE)
                        if di >= 0:
                            nc.vector.tensor_mul(pt[:, :P], pt[:, :P],
                                                 cmask[:, 0, :P])
                        nc.tensor.matmul(psO_t[:, c0:], lhsT=vh[:, kt, :],
                                         rhs=pt[:, :w],
                                         start=(kt == 0), stop=(kt == nkt - 1),
                                         skip_group_check=True)
                        # fp32 running key-tile sum on DVE (hidden behind the
                        # exp pacing); one ones-matmul at the end reduces
                        # across partitions and broadcasts
                        if kt == 0:
                            nc.vector.tensor_copy(out=racc, in_=pt)
                        else:
                            nc.vector.tensor_add(racc[:, c0:], racc[:, c0:],
                                                 pt[:, :w])
                        pump_fills(1 + (kt & 1))
                    raccb = work.tile([P, 512], BF16, name="raccb", tag="raccb")
                    nc.vector.tensor_copy(out=raccb, in_=racc)
                    psR_t = psR.tile([P, 512], F32, name="psrow", tag="psr")
                    nc.tensor.matmul(psR_t, lhsT=ones, rhs=raccb,
                                     start=True, stop=True)
                    rec = work.tile([P, 512], F32, name="rec", tag="rec")
                    nc.vector.reciprocal_approx_fast(out=rec, in_=psR_t)
                    nc.vector.tensor_mul(ot[h][:, qb * 512:(qb + 1) * 512],
                                         psO_t, rec)
                    if h == H - 1:
                        # this token block's ot columns are now complete for
                        # every head: its o_proj chains become fill work
                        fills.append(oproj_nb_gen(qb))
                    pump_fills(4)
                if h < H - 1:
                    # finish next head's projections before its attention
                    while fills:
                        pump_fills(64)
                cur_tiles = next_tiles
            # drain remaining o_proj work
            while fills:
                pump_fills(64)

    nc.compile()
    return nc


def _bitlinear_eff(w):
    """Forward-effective BitLinear weight: clip(round(w/gamma),-1,1)*gamma."""
    w = np.asarray(w, dtype=np.float32)
    gamma = max(np.float32(np.abs(w).mean()), np.float32(1e-5))
    q = np.clip(np.round(w / gamma), -1.0, 1.0).astype(np.float32)
    return q * gamma


def _causal_masks():
    k = np.arange(128)[:, None]
    q = np.arange(512)[None, :]
    m = np.stack([(k <= q - 128 * i) for i in range(4)]).astype(np.float32)
    return m.astype(ml_dtypes.bfloat16)


def _tile_qkv(w_shard):
    """[F, D] -> [H, 128, KD*128]: [h, p, kd*128+f] = w_shard[h*128+f, kd*128+p]."""
    Fs, Ds = w_shard.shape
    a = w_shard.reshape(Fs // 128, 128, Ds // 128, 128)  # [h, f, kd, p]
    a = a.transpose(0, 3, 2, 1).reshape(Fs // 128, 128, Ds)
    return np.ascontiguousarray(a)


def _tile_wo(wo_shard):
    """[D, F] -> [MT, 128, H*128]: [m, p, h*128+j] = wo_shard[m*128+j, h*128+p]."""
    Ds, Fs = wo_shard.shape
    a = wo_shard.reshape(Ds // 128, 128, Fs // 128, 128)  # [m, j, h, p]
    a = a.transpose(0, 3, 2, 1).reshape(Ds // 128, 128, Fs)
    return np.ascontiguousarray(a)


def _prep_inputs(x, wq, wk, wv, wo):
    bf = ml_dtypes.bfloat16
    x = np.asarray(x, dtype=np.float32)
    effs = {n: _bitlinear_eff(w) for n, w in
            (("wq", wq), ("wk", wk), ("wv", wv), ("wo", wo))}
    cmask = _causal_masks()
    xTs = [np.ascontiguousarray(x[b].T).astype(bf) for b in range(B)]
    shards = []
    for hg in range(2):
        rows = slice(hg * F_LOC, (hg + 1) * F_LOC)
        shards.append({
            "wqT": _tile_qkv(effs["wq"][rows, :]).astype(bf),
            "wkT": _tile_qkv(effs["wk"][rows, :]).astype(bf),
            "wvT": _tile_qkv(effs["wv"][rows, :]).astype(bf),
            "woT": _tile_wo(effs["wo"][:, rows]).astype(bf),
        })
    in_maps = []
    for c in range(N_CORES):
        b, hg = c // 2, c % 2
        m = {"xT": xTs[b], "cmask": cmask}
        m.update(shards[hg])
        in_maps.append(m)
    return in_maps


_NC_CACHE = {}


def _get_nc():
    if "nc" not in _NC_CACHE:
        _NC_CACHE["nc"] = build_bass()
    return _NC_CACHE["nc"]


def run(x, wq, wk, wv, wo, trace=False):
    nc = _get_nc()
    in_maps = _prep_inputs(x, wq, wk, wv, wo)
    res = bass_utils.run_bass_kernel_spmd(
        nc, in_maps, core_ids=list(range(N_CORES)), trace=trace)
    out = np.empty((B, T_FULL, D_MODEL), dtype=np.float32)
    for b in range(B):
        out[b] = (res.results[2 * b]["outT"]
                  + res.results[2 * b + 1]["outT"]).T
    return out, res


def kernel(x, wq, wk, wv, wo):
    out, _ = run(x, wq, wk, wv, wo)
    return out



# revision 3
# speedup vs baseline: 1.0580x; 1.0580x over previous
"""BitSelfAttention on 8 TRN2 NeuronCores — fp8 DoubleRow version.

Sharding: core c handles batch b = c//2 and head-group hg = c%2 (8 of 16 heads).
Each core computes its 8 heads' QKV projections + causal attention + its slice
of the o_proj GEMM, producing a partial output (transposed, [D, T], fp32).
Host pre-quantizes BitLinear weights to TERNARY {-1,0,+1} (exact in fp8/bf16)
and folds the gammas out: gamma_q*gamma_k/sqrt(dh) rides the exp() scale
operand on device; gamma_v*gamma_o is applied on host to the final output.

Precision plan (validated by numpy simulation against the 2e-2 gate):
the output error is dominated by EARLY tokens (short causal rows average few
v's: token 0's attention output is exactly v[0]), so the first 512-token
block runs bf16 end-to-end while blocks 1-3 run fp8 e4m3 with DoubleRow
matmuls (2 fp8 contraction rows per PE cell, ~1.8x the bf16 matmul rate):
  - Q/K/V projections: token block 0 bf16, blocks 1-3 fp8 DoubleRow.
  - V is projected in TRANSPOSED orientation (x tile stationary, weight tile
    moving) producing token-major [tok, dh] tiles directly - no PE transposes.
  - S^T matmul: bf16 always (contraction dh=128 cannot pair).
  - exp: query block 0 -> bf16 P; blocks 1-3 -> fp8 P.
  - AV: qb0 bf16 singles; qb>=1 full key tiles as fp8 DoubleRow pairs,
    diagonal tiles as fp8 normal-rate singles (narrowed to live columns).
  - o_proj: token block 0 bf16, blocks 1-3 fp8 DoubleRow over head pairs.
Softmax row sums accumulate in fp32 on DVE per key tile; one all-ones bf16
matmul per block reduces across partitions (keys) and broadcasts; normalize
by fast-reciprocal multiply during PSUM eviction. Cross-head fill
interleaving as in the bf16 kernel: the next head's projection chains (and
the last head's per-block o_proj chains) are pumped between attention ops so
the in-order PE always has work while exp results are in flight.
"""

import math

import ml_dtypes
import numpy as np

import concourse.mybir as mybir
import concourse.tile as tile
from concourse import bacc
from concourse import bass_utils

BF16 = mybir.dt.bfloat16
F32 = mybir.dt.float32
F8 = mybir.dt.float8e4
E4 = ml_dtypes.float8_e4m3
DR = mybir.MatmulPerfMode.DoubleRow

D_MODEL = 2048
N_HEAD = 16
D_HEAD = 128
B = 4
T_FULL = 2048
N_CORES = 8
F_LOC = D_MODEL // 2  # features per core (8 heads)


def build_bass(T=T_FULL, D=D_MODEL, F=F_LOC, debug=False):
    """Build the single-core program (SPMD across 8 cores via input data)."""
    P = 128
    KD = D // P      # contraction 128-tiles
    TT = T // P      # token 128-tiles
    TB = T // 512    # token 512-blocks
    H = F // P       # local heads
    MT = D // P      # output-dmodel 128-tiles
    KT_PER_B = 512 // P

    nc = bacc.Bacc("TRN2", target_bir_lowering=False, debug=debug,
                   num_devices=N_CORES)
    xT8_d = nc.dram_tensor("xT8", [D, T], F8, kind="ExternalInput").ap()
    xT16_d = nc.dram_tensor("xT16", [D, 512], BF16, kind="ExternalInput").ap()
    # ternary weights pre-tiled on host (contiguous DMAs):
    #   wq/wk/wv: [H, 128, KD*128] with [h, p, kd*128+f] = w_q[h*128+f, kd*128+p]
    #   wo:       [MT, 128, H*128] with [m, p, h*128+j] = w_o[m*128+j, h*128+p]
    wq8_d = nc.dram_tensor("wq8", [H, P, KD * P], F8, kind="ExternalInput").ap()
    wk8_d = nc.dram_tensor("wk8", [H, P, KD * P], F8, kind="ExternalInput").ap()
    wv8_d = nc.dram_tensor("wv8", [H, P, KD * P], F8, kind="ExternalInput").ap()
    wq16_d = nc.dram_tensor("wq16", [H, P, KD * P], BF16,
                            kind="ExternalInput").ap()
    wk16_d = nc.dram_tensor("wk16", [H, P, KD * P], BF16,
                            kind="ExternalInput").ap()
    wv16_d = nc.dram_tensor("wv16", [H, P, KD * P], BF16,
                            kind="ExternalInput").ap()
    wo8_d = nc.dram_tensor("wo8", [MT, P, H * P], F8, kind="ExternalInput").ap()
    wo16_d = nc.dram_tensor("wo16", [MT, P, H * P], BF16,
                            kind="ExternalInput").ap()
    cm8_d = nc.dram_tensor("cm8", [P, P], F8, kind="ExternalInput").ap()
    cm16_d = nc.dram_tensor("cm16", [P, P], BF16, kind="ExternalInput").ap()
    qsc_d = nc.dram_tensor("qsc", [P, 1], F32, kind="ExternalInput").ap()
    out_d = nc.dram_tensor("outT", [D, T], F32, kind="ExternalOutput").ap()

    with tile.TileContext(nc) as tc:
        with (
            tc.tile_pool(name="big", bufs=1) as big,
            tc.tile_pool(name="work", bufs=2) as work,
            tc.tile_pool(name="psS", bufs=3, space="PSUM") as psS,
            tc.tile_pool(name="psO", bufs=2, space="PSUM") as psO,
            tc.tile_pool(name="psR", bufs=1, space="PSUM") as psR,
            tc.tile_pool(name="psP", bufs=2, space="PSUM") as psP,
        ):
            # ---- persistent inputs (head-0 V weights first: first MMs)
            wvh16_0 = work.tile([P, KD, P], BF16, name="wvh16_0", tag="wvh16")
            nc.sync.dma_start(out=wvh16_0.rearrange("p kd f -> p (kd f)"),
                              in_=wv16_d[0])
            xb16 = big.tile([P, KD, 512], BF16, name="xb16", tag="xb16", bufs=1)
            for kd in range(KD):
                nc.sync.dma_start(out=xb16[:, kd, :],
                                  in_=xT16_d[kd * P:(kd + 1) * P, :])
            wvh8_0 = work.tile([P, KD, P], F8, name="wvh8_0", tag="wvh8")
            nc.sync.dma_start(out=wvh8_0.rearrange("p kd f -> p (kd f)"),
                              in_=wv8_d[0])
            xbig = big.tile([P, KD, T], F8, name="xbig", tag="xbig", bufs=1)
            for kd in range(KD):
                nc.sync.dma_start(out=xbig[:, kd, :],
                                  in_=xT8_d[kd * P:(kd + 1) * P, :])
            ones = big.tile([P, P], BF16, name="ones_sb", tag="ones", bufs=1)
            nc.vector.memset(ones, 1.0)
            cm8 = big.tile([P, P], F8, name="cm8_sb", tag="cm8", bufs=1)
            nc.sync.dma_start(out=cm8, in_=cm8_d)
            cm16 = big.tile([P, P], BF16, name="cm16_sb", tag="cm16", bufs=1)
            nc.sync.dma_start(out=cm16, in_=cm16_d)
            qsc = big.tile([P, 1], F32, name="qsc_sb", tag="qsc", bufs=1)
            nc.sync.dma_start(out=qsc, in_=qsc_d)
            # attention outputs: block 0 bf16, blocks 1-3 fp8 (unscaled v-hat)
            ot016 = big.tile([P, H, 512], BF16, name="ot016", tag="ot016",
                             bufs=1)
            ot8 = big.tile([P, H, T - 512], F8, name="ot8", tag="ot8", bufs=1)

            def load_head_weights(h, wvh16=None, wvh8=None):
                if wvh16 is None:
                    wvh16 = work.tile([P, KD, P], BF16, name=f"wvh16_{h}",
                                      tag="wvh16")
                    nc.sync.dma_start(
                        out=wvh16.rearrange("p kd f -> p (kd f)"),
                        in_=wv16_d[h])
                if wvh8 is None:
                    wvh8 = work.tile([P, KD, P], F8, name=f"wvh8_{h}",
                                     tag="wvh8")
                    nc.sync.dma_start(out=wvh8.rearrange("p kd f -> p (kd f)"),
                                      in_=wv8_d[h])
                wqh8 = work.tile([P, KD, P], F8, name=f"wqh8_{h}", tag="wqh8")
                nc.sync.dma_start(out=wqh8.rearrange("p kd f -> p (kd f)"),
                                  in_=wq8_d[h])
                wkh8 = work.tile([P, KD, P], F8, name=f"wkh8_{h}", tag="wkh8")
                nc.sync.dma_start(out=wkh8.rearrange("p kd f -> p (kd f)"),
                                  in_=wk8_d[h])
                wqh16 = work.tile([P, KD, P], BF16, name=f"wqh16_{h}",
                                  tag="wqh16")
                nc.sync.dma_start(out=wqh16.rearrange("p kd f -> p (kd f)"),
                                  in_=wq16_d[h])
                wkh16 = work.tile([P, KD, P], BF16, name=f"wkh16_{h}",
                                  tag="wkh16")
                nc.sync.dma_start(out=wkh16.rearrange("p kd f -> p (kd f)"),
                                  in_=wk16_d[h])
                return wqh8, wkh8, wvh8, wqh16, wkh16, wvh16

            def alloc_head_tiles(h):
                qt_ = work.tile([P, T], BF16, name=f"qt{h}", tag="qt")
                kt_ = work.tile([P, T], BF16, name=f"kt{h}", tag="kt")
                vh8 = work.tile([P, TT, P], F8, name=f"vh8_{h}", tag="vh8")
                vh16 = work.tile([P, KT_PER_B, P], BF16, name=f"vh16_{h}",
                                 tag="vh16")
                return qt_, kt_, vh8, vh16

            def proj_fill_gen(ws, tiles):
                """V (transposed, token-major out) then Q then K projection
                chains, yielding after every matmul."""
                wqh8, wkh8, wvh8, wqh16, wkh16, wvh16 = ws
                qt_, kt_, vh8, vh16 = tiles
                # V: 16 token-tile chains; out partitions = tokens.
                # Each chain gets its own PSUM tile: start_tensor_calc
                # clears the whole 2KB bank (ZERO_REGION), so sub-regions
                # of one bank cannot host staggered accumulation groups.
                for tt in range(TT):
                    psv = psP.tile([P, 512], F32, name=f"psv{tt}", tag="psp")
                    sub = psv[:, 0:P]
                    ts_ = slice(tt * P, (tt + 1) * P)
                    if tt < KT_PER_B:  # block-0 tokens: bf16
                        for kd in range(KD):
                            nc.tensor.matmul(sub, lhsT=xb16[:, kd, ts_],
                                             rhs=wvh16[:, kd, :],
                                             start=(kd == 0),
                                             stop=(kd == KD - 1))
                            yield
                    else:
                        for j in range(KD // 2):
                            nc.tensor.matmul(sub,
                                             lhsT=xbig[:, 2 * j:2 * j + 2, ts_],
                                             rhs=wvh8[:, 2 * j:2 * j + 2, :],
                                             start=(j == 0),
                                             stop=(j == KD // 2 - 1),
                                             perf_mode=DR)
                            yield
                    nc.vector.tensor_copy(out=vh8[:, tt, :], in_=sub)
                    if tt < KT_PER_B:
                        nc.vector.tensor_copy(out=vh16[:, tt, :], in_=sub)
                # Q then K: dh-major out [dh, tokens]
                for w8, w16, dst in ((wqh8, wqh16, qt_), (wkh8, wkh16, kt_)):
                    for tb in range(TB):
                        ts_ = slice(tb * 512, (tb + 1) * 512)
                        ps = psP.tile([P, 512], F32, name="psfill", tag="psp")
                        if tb == 0:  # block-0 q,k: bf16
                            for kd in range(KD):
                                nc.tensor.matmul(ps, lhsT=w16[:, kd, :],
                                                 rhs=xb16[:, kd, :],
                                                 start=(kd == 0),
                                                 stop=(kd == KD - 1))
                                yield
                        else:
                            for j in range(KD // 2):
                                nc.tensor.matmul(
                                    ps, lhsT=w8[:, 2 * j:2 * j + 2, :],
                                    rhs=xbig[:, 2 * j:2 * j + 2, ts_],
                                    start=(j == 0),
                                    stop=(j == KD // 2 - 1),
                                    perf_mode=DR)
                                yield
                        nc.vector.tensor_copy(out=dst[:, ts_], in_=ps)

            def pump(gen, n):
                for _ in range(n):
                    try:
                        next(gen)
                    except StopIteration:
                        return False
                return True

            def pump_n(gen, n):
                c = 0
                for _ in range(n):
                    try:
                        next(gen)
                        c += 1
                    except StopIteration:
                        break
                return c

            def oproj_nb_gen(nb):
                """o_proj chains for one token block, yielding per matmul."""
                ns = slice(nb * 512, (nb + 1) * 512)
                for m in range(MT):
                    if nb == 0:
                        woh = work.tile([P, H, P], BF16, name=f"woh16_{m}",
                                        tag="woh16", bufs=4)
                        nc.sync.dma_start(
                            out=woh.rearrange("p h f -> p (h f)"),
                            in_=wo16_d[m])
                        yield  # cover the woh DMA latency
                        ps = psP.tile([P, 512], F32, name="psout", tag="psp")
                        for hh in range(H):
                            nc.tensor.matmul(ps, lhsT=woh[:, hh, :],
                                             rhs=ot016[:, hh, :],
                                             start=(hh == 0),
                                             stop=(hh == H - 1))
                            yield
                    else:
                        woh = work.tile([P, H, P], F8, name=f"woh8_{m}",
                                        tag="woh8", bufs=4)
                        nc.sync.dma_start(
                            out=woh.rearrange("p h f -> p (h f)"),
                            in_=wo8_d[m])
                        yield
                        ps = psP.tile([P, 512], F32, name="psout", tag="psp")
                        os_ = slice((nb - 1) * 512, nb * 512)
                        for i in range(H // 2):
                            nc.tensor.matmul(
                                ps, lhsT=woh[:, 2 * i:2 * i + 2, :],
                                rhs=ot8[:, 2 * i:2 * i + 2, os_],
                                start=(i == 0), stop=(i == H // 2 - 1),
                                perf_mode=DR)
                            yield
                    stg = work.tile([P, 512], F32, name="ostage", tag="ostage",
                                    bufs=4)
                    nc.vector.tensor_copy(out=stg, in_=ps)
                    nc.sync.dma_start(out=out_d[m * P:(m + 1) * P, ns],
                                      in_=stg)

            # head-0 weights load after x tiles; head-1 after that
            ws_list = [None] * (H + 2)
            ws_list[0] = load_head_weights(0, wvh16=wvh16_0, wvh8=wvh8_0)
            if H > 1:
                ws_list[1] = load_head_weights(1)
            cur_tiles = alloc_head_tiles(0)
            g0 = proj_fill_gen(ws_list[0], cur_tiles)
            while pump(g0, 1):
                pass

            fills = []

            def pump_fills(n):
                while n > 0 and fills:
                    n -= pump_n(fills[0], n)
                    if n > 0:
                        fills.pop(0)

            for h in range(H):
                qt_, kt_, vh8, vh16 = cur_tiles
                # prefetch weights two heads ahead
                if h + 2 < H:
                    ws_list[h + 2] = load_head_weights(h + 2)
                if h + 1 < H:
                    next_tiles = alloc_head_tiles(h + 1)
                    fills.append(proj_fill_gen(ws_list[h + 1], next_tiles))
                else:
                    next_tiles = None

                # causal attention, S^T layout (keys on partitions).
                # qb0: all-bf16 P/V path. qb>=1: fp8; full key tiles pair
                # into DoubleRow AV matmuls, diagonal tiles are narrowed
                # singles. Only the first 128 live columns of a diagonal
                # tile are triangular -> one [128,128] mask multiply.
                for qb in range(TB):
                    nkt = KT_PER_B * (qb + 1)
                    psO_t = psO.tile([P, 512], F32, name="psodt", tag="pso")
                    racc = work.tile([P, 512], F32, name="racc", tag="racc")
                    ptp = None
                    for kt in range(nkt):
                        di = kt - KT_PER_B * qb
                        c0 = max(di, 0) * P  # first live query column
                        w = 512 - c0
                        qs = slice(qb * 512 + c0, (qb + 1) * 512)
                        psS_t = psS.tile([P, 512], F32, name="pssc", tag="pss")
                        nc.tensor.matmul(psS_t[:, :w],
                                         lhsT=kt_[:, kt * P:(kt + 1) * P],
                                         rhs=qt_[:, qs],
                                         start=True, stop=True)
                        if qb == 0:
                            pt = work.tile([P, 512], BF16, name="pt16",
                                           tag="pt16", bufs=4)
                            pts = pt[:, :w]
                        elif di < 0:  # paired full tiles
                            if kt % 2 == 0:
                                ptp = work.tile([P, 2, 512], F8, name="ptp8",
                                                tag="ptp8", bufs=3)
                            pts = ptp[:, kt % 2, :]
                        else:  # diagonal singles
                            pt = work.tile([P, 512], F8, name="ptd8",
                                           tag="ptd8", bufs=4)
                            pts = pt[:, :w]
                        nc.scalar.activation(
                            out=pts, in_=psS_t[:, :w],
                            func=mybir.ActivationFunctionType.Exp, scale=qsc)
                        if di >= 0:
                            nc.vector.tensor_mul(pts[:, :P], pts[:, :P],
                                                 cm16 if qb == 0 else cm8)
                        # fp32 running key-tile sum on DVE
                        if kt == 0:
                            nc.vector.tensor_copy(out=racc, in_=pts)
                        else:
                            nc.vector.tensor_add(racc[:, c0:], racc[:, c0:],
                                                 pts)
                        # AV
                        if qb == 0:
                            nc.tensor.matmul(psO_t[:, c0:],
                                             lhsT=vh16[:, kt, :], rhs=pts,
                                             start=(kt == 0),
                                             stop=(kt == nkt - 1),
                                             skip_group_check=True)
                        elif di < 0:
                            if kt % 2 == 1:
                                nc.tensor.matmul(
                                    psO_t, lhsT=vh8[:, kt - 1:kt + 1, :],
                                    rhs=ptp, start=(kt == 1), stop=False,
                                    perf_mode=DR, skip_group_check=True)
                        else:
                            nc.tensor.matmul(psO_t[:, c0:],
                                             lhsT=vh8[:, kt, :], rhs=pts,
                                             start=False,
                                             stop=(kt == nkt - 1),
                                             skip_group_check=True)
                        pump_fills(2 + (kt & 1))
                    raccb = work.tile([P, 512], BF16, name="raccb",
                                      tag="raccb")
                    nc.vector.tensor_copy(out=raccb, in_=racc)
                    psR_t = psR.tile([P, 512], F32, name="psrow", tag="psr")
                    nc.tensor.matmul(psR_t, lhsT=ones, rhs=raccb,
                                     start=True, stop=True)
                    rec = work.tile([P, 512], F32, name="rec", tag="rec")
                    nc.vector.reciprocal_approx_fast(out=rec, in_=psR_t)
                    if qb == 0:
                        nc.vector.tensor_mul(ot016[:, h, :], psO_t, rec)
                    else:
                        nc.vector.tensor_mul(
                            ot8[:, h, (qb - 1) * 512:qb * 512], psO_t, rec)
                    if h == H - 1:
                        # this token block's ot columns are complete for
                        # every head: its o_proj chains become fill work
                        fills.append(oproj_nb_gen(qb))
                    pump_fills(4)
                if h < H - 1:
                    # finish next head's projections before its attention
                    while fills:
                        pump_fills(64)
                cur_tiles = next_tiles
            # drain remaining o_proj work
            while fills:
                pump_fills(64)

    nc.compile()
    return nc


def _bit_ternary(w):
    """Ternary BitLinear weight and its gamma: w_eff = q * gamma."""
    w = np.asarray(w, dtype=np.float32)
    gamma = max(np.float32(np.abs(w).mean(dtype=np.float32)), np.float32(1e-5))
    q = np.clip(np.round(w / gamma), -1.0, 1.0).astype(np.float32)
    return q, gamma


def _causal_mask():
    k = np.arange(128)[:, None]
    q = np.arange(128)[None, :]
    return (k <= q).astype(np.float32)


def _tile_qkv(w_shard):
    """[F, D] -> [H, 128, KD*128]: [h, p, kd*128+f] = w_shard[h*128+f, kd*128+p]."""
    Fs, Ds = w_shard.shape
    a = w_shard.reshape(Fs // 128, 128, Ds // 128, 128)  # [h, f, kd, p]
    a = a.transpose(0, 3, 2, 1).reshape(Fs // 128, 128, Ds)
    return np.ascontiguousarray(a)


def _tile_wo(wo_shard):
    """[D, F] -> [MT, 128, H*128]: [m, p, h*128+j] = wo_shard[m*128+j, h*128+p]."""
    Ds, Fs = wo_shard.shape
    a = wo_shard.reshape(Ds // 128, 128, Fs // 128, 128)  # [m, j, h, p]
    a = a.transpose(0, 3, 2, 1).reshape(Ds // 128, 128, Fs)
    return np.ascontiguousarray(a)


def _prep_inputs(x, wq, wk, wv, wo):
    bf = ml_dtypes.bfloat16
    x = np.asarray(x, dtype=np.float32)
    tern = {}
    gam = {}
    for n, w in (("wq", wq), ("wk", wk), ("wv", wv), ("wo", wo)):
        tern[n], gam[n] = _bit_ternary(w)
    cm = _causal_mask()
    qsc = np.full((128, 1),
                  gam["wq"] * gam["wk"] / np.float32(math.sqrt(D_HEAD)),
                  np.float32)
    xT8s, xT16s = [], []
    for b in range(B):
        xt = np.ascontiguousarray(x[b].T)
        xT8s.append(xt.astype(E4))
        xT16s.append(np.ascontiguousarray(xt[:, :512]).astype(bf))
    shards = []
    for hg in range(2):
        rows = slice(hg * F_LOC, (hg + 1) * F_LOC)
        tq = _tile_qkv(tern["wq"][rows, :])
        tk = _tile_qkv(tern["wk"][rows, :])
        tv = _tile_qkv(tern["wv"][rows, :])
        to = _tile_wo(tern["wo"][:, rows])
        shards.append({
            "wq8": tq.astype(E4), "wk8": tk.astype(E4), "wv8": tv.astype(E4),
            "wq16": tq.astype(bf), "wk16": tk.astype(bf),
            "wv16": tv.astype(bf),
            "wo8": to.astype(E4), "wo16": to.astype(bf),
        })
    in_maps = []
    for c in range(N_CORES):
        b, hg = c // 2, c % 2
        m = {"xT8": xT8s[b], "xT16": xT16s[b], "cm8": cm.astype(E4),
             "cm16": cm.astype(bf), "qsc": qsc}
        m.update(shards[hg])
        in_maps.append(m)
    return in_maps, np.float32(gam["wv"] * gam["wo"])


_NC_CACHE = {}


def _get_nc():
    if "nc" not in _NC_CACHE:
        _NC_CACHE["nc"] = build_bass()
    return _NC_CACHE["nc"]


def run(x, wq, wk, wv, wo, trace=False):
    nc = _get_nc()
    in_maps, oscale = _prep_inputs(x, wq, wk, wv, wo)
    res = bass_utils.run_bass_kernel_spmd(
        nc, in_maps, core_ids=list(range(N_CORES)), trace=trace)
    out = np.empty((B, T_FULL, D_MODEL), dtype=np.float32)
    for b in range(B):
        out[b] = (res.results[2 * b]["outT"]
                  + res.results[2 * b + 1]["outT"]).T * oscale
    return out, res


def kernel(x, wq, wk, wv, wo):
    out, _ = run(x, wq, wk, wv, wo)
    return out


# revision 4
# speedup vs baseline: 1.2153x; 1.1486x over previous
"""BitSelfAttention on 8 TRN2 NeuronCores — fp8 DoubleRow version.

Sharding: core c handles batch b = c//2 and head-group hg = c%2 (8 of 16 heads).
Each core computes its 8 heads' QKV projections + causal attention + its slice
of the o_proj GEMM, producing a partial output (transposed, [D, T], fp32).
Host pre-quantizes BitLinear weights to TERNARY {-1,0,+1} (exact in fp8/bf16)
and folds the gammas out: gamma_q*gamma_k/sqrt(dh) rides the exp() scale
operand on device; gamma_v*gamma_o is applied on host to the final output.

Precision plan (validated by numpy simulation against the 2e-2 gate):
the output error is dominated by EARLY tokens (short causal rows average few
v's: token 0's attention output is exactly v[0]), so the first 512-token
block runs bf16 end-to-end while blocks 1-3 run fp8 e4m3 with DoubleRow
matmuls (2 fp8 contraction rows per PE cell, ~1.8x the bf16 matmul rate):
  - Q/K projections: token block 0 bf16, blocks 1-3 fp8 DoubleRow.
  - V is projected in TRANSPOSED orientation (x tile stationary, weights
    moving) producing token-major [tok, 4*dh] tiles directly — no PE
    transposes. Four heads share one chain so the moving operand is 512
    wide and the 256-column DoubleRow LDWEIGHTS stays hidden.
  - S^T matmul: bf16 always (contraction dh=128 cannot pair).
  - exp: query block 0 -> bf16 P; blocks 1-3 -> fp8 P.
  - AV: qb0 bf16 singles; qb>=1 full key tiles as fp8 DoubleRow pairs,
    diagonal tiles as fp8 normal-rate singles (narrowed to live columns).
  - o_proj: token block 0 bf16, blocks 1-3 fp8 DoubleRow over head pairs.
Softmax denominators accumulate on the PE: an all-ones stationary matmul per
key tile (DoubleRow-paired where P is paired) adds the partition-reduced
row sums into a dedicated PSUM bank, already broadcast across partitions;
normalize by fast-reciprocal multiply during PSUM eviction. Cross-head fill
interleaving as in the bf16 kernel: the next head's projection chains (and
the last head's per-block o_proj chains) are pumped between attention ops so
the in-order PE always has work while exp results are in flight.
"""

import math

import ml_dtypes
import numpy as np

import concourse.mybir as mybir
import concourse.tile as tile
from concourse import bacc
from concourse import bass_utils

BF16 = mybir.dt.bfloat16
F32 = mybir.dt.float32
F8 = mybir.dt.float8e4
E4 = ml_dtypes.float8_e4m3
DR = mybir.MatmulPerfMode.DoubleRow

D_MODEL = 2048
N_HEAD = 16
D_HEAD = 128
B = 4
T_FULL = 2048
N_CORES = 8
F_LOC = D_MODEL // 2  # features per core (8 heads)


def build_bass(T=T_FULL, D=D_MODEL, F=F_LOC, debug=False):
    """Build the single-core program (SPMD across 8 cores via input data)."""
    P = 128
    KD = D // P      # contraction 128-tiles
    TT = T // P      # token 128-tiles
    TB = T // 512    # token 512-blocks
    H = F // P       # local heads
    MT = D // P      # output-dmodel 128-tiles
    KT_PER_B = 512 // P
    G = H // 4       # 4-head V groups

    nc = bacc.Bacc("TRN2", target_bir_lowering=False, debug=debug,
                   num_devices=N_CORES)
    xT8_d = nc.dram_tensor("xT8", [D, T], F8, kind="ExternalInput").ap()
    xT16_d = nc.dram_tensor("xT16", [D, 512], BF16, kind="ExternalInput").ap()
    # ternary weights pre-tiled on host (contiguous DMAs):
    #   wq/wk: [H, 128, KD*128] with [h, p, kd*128+f] = w[h*128+f, kd*128+p]
    #   wv:    [G, 128, KD*512] with [g, p, kd*512+hh*128+f]
    #            = wv[(4g+hh)*128+f, kd*128+p]
    #   wo:    [MT, 128, H*128] with [m, p, h*128+j] = wo[m*128+j, h*128+p]
    wq8_d = nc.dram_tensor("wq8", [H, P, KD * P], F8, kind="ExternalInput").ap()
    wk8_d = nc.dram_tensor("wk8", [H, P, KD * P], F8, kind="ExternalInput").ap()
    wq16_d = nc.dram_tensor("wq16", [H, P, KD * P], BF16,
                            kind="ExternalInput").ap()
    wk16_d = nc.dram_tensor("wk16", [H, P, KD * P], BF16,
                            kind="ExternalInput").ap()
    wv8_d = nc.dram_tensor("wv8", [G, P, KD * 512], F8,
                           kind="ExternalInput").ap()
    wv16_d = nc.dram_tensor("wv16", [G, P, KD * 512], BF16,
                            kind="ExternalInput").ap()
    wo8_d = nc.dram_tensor("wo8", [MT, P, H * P], F8, kind="ExternalInput").ap()
    wo16_d = nc.dram_tensor("wo16", [MT, P, H * P], BF16,
                            kind="ExternalInput").ap()
    cm8_d = nc.dram_tensor("cm8", [P, P], F8, kind="ExternalInput").ap()
    cm16_d = nc.dram_tensor("cm16", [P, P], BF16, kind="ExternalInput").ap()
    qsc_d = nc.dram_tensor("qsc", [P, 1], F32, kind="ExternalInput").ap()
    out_d = nc.dram_tensor("outT", [D, T], F32, kind="ExternalOutput").ap()

    with tile.TileContext(nc) as tc:
        with (
            tc.tile_pool(name="big", bufs=1) as big,
            tc.tile_pool(name="work", bufs=2) as work,
            tc.tile_pool(name="psS", bufs=2, space="PSUM") as psS,
            tc.tile_pool(name="psO", bufs=2, space="PSUM") as psO,
            tc.tile_pool(name="psR", bufs=2, space="PSUM") as psR,
            tc.tile_pool(name="psP", bufs=2, space="PSUM") as psP,
        ):
            # ---- persistent inputs, ordered so the first V chains (bf16,
            # group 0) can start while the fp8 x image still streams in.
            wv16g0 = work.tile([P, KD, 512], BF16, name="wv16g0", tag="wv16g",
                               bufs=1)
            nc.sync.dma_start(out=wv16g0.rearrange("p kd f -> p (kd f)"),
                              in_=wv16_d[0])
            xb16 = big.tile([P, KD, 512], BF16, name="xb16", tag="xb16", bufs=1)
            for kd in range(KD):
                nc.sync.dma_start(out=xb16[:, kd, :],
                                  in_=xT16_d[kd * P:(kd + 1) * P, :])
            ones = big.tile([P, P], BF16, name="ones_sb", tag="ones", bufs=1)
            nc.vector.memset(ones, 1.0)
            ones8 = big.tile([P, 2, P], F8, name="ones8_sb", tag="ones8",
                             bufs=1)
            nc.vector.memset(ones8, 1.0)
            cm8 = big.tile([P, P], F8, name="cm8_sb", tag="cm8", bufs=1)
            nc.sync.dma_start(out=cm8, in_=cm8_d)
            cm16 = big.tile([P, P], BF16, name="cm16_sb", tag="cm16", bufs=1)
            nc.sync.dma_start(out=cm16, in_=cm16_d)
            qsc = big.tile([P, 1], F32, name="qsc_sb", tag="qsc", bufs=1)
            nc.sync.dma_start(out=qsc, in_=qsc_d)
            wv8g0 = work.tile([P, KD, 512], F8, name="wv8g0", tag="wv8g",
                              bufs=1)
            nc.sync.dma_start(out=wv8g0.rearrange("p kd f -> p (kd f)"),
                              in_=wv8_d[0])
            xbig = big.tile([P, KD, T], F8, name="xbig", tag="xbig", bufs=1)
            for kd in range(KD):
                nc.sync.dma_start(out=xbig[:, kd, :],
                                  in_=xT8_d[kd * P:(kd + 1) * P, :])
            # attention outputs: block 0 bf16, blocks 1-3 fp8 (unscaled v-hat)
            ot016 = big.tile([P, H, 512], BF16, name="ot016", tag="ot016",
                             bufs=1)
            ot8 = big.tile([P, H, T - 512], F8, name="ot8", tag="ot8", bufs=1)

            def load_qk_weights(h):
                wqh8 = work.tile([P, KD, P], F8, name=f"wqh8_{h}", tag="wqh8")
                nc.sync.dma_start(out=wqh8.rearrange("p kd f -> p (kd f)"),
                                  in_=wq8_d[h])
                wkh8 = work.tile([P, KD, P], F8, name=f"wkh8_{h}", tag="wkh8")
                nc.sync.dma_start(out=wkh8.rearrange("p kd f -> p (kd f)"),
                                  in_=wk8_d[h])
                wqh16 = work.tile([P, KD, P], BF16, name=f"wqh16_{h}",
                                  tag="wqh16")
                nc.sync.dma_start(out=wqh16.rearrange("p kd f -> p (kd f)"),
                                  in_=wq16_d[h])
                wkh16 = work.tile([P, KD, P], BF16, name=f"wkh16_{h}",
                                  tag="wkh16")
                nc.sync.dma_start(out=wkh16.rearrange("p kd f -> p (kd f)"),
                                  in_=wk16_d[h])
                return wqh8, wkh8, wqh16, wkh16

            def load_v_weights(g, w16=None, w8=None):
                if w16 is None:
                    w16 = work.tile([P, KD, 512], BF16, name=f"wv16g{g}",
                                    tag="wv16g", bufs=1)
                    nc.sync.dma_start(out=w16.rearrange("p kd f -> p (kd f)"),
                                      in_=wv16_d[g])
                if w8 is None:
                    w8 = work.tile([P, KD, 512], F8, name=f"wv8g{g}",
                                   tag="wv8g", bufs=1)
                    nc.sync.dma_start(out=w8.rearrange("p kd f -> p (kd f)"),
                                      in_=wv8_d[g])
                return w16, w8

            def alloc_v_tiles(g):
                vh8 = work.tile([P, TT, 512], F8, name=f"vh8g{g}", tag="vh8g")
                vh16 = work.tile([P, KT_PER_B, 512], BF16, name=f"vh16g{g}",
                                 tag="vh16g")
                return vh8, vh16

            def vgroup_gen(vws, vtiles):
                """Transposed V projection for a 4-head group: token-major
                [tok, 4*dh] tiles, one chain per token tile."""
                wv16g, wv8g = vws
                vh8, vh16 = vtiles
                for tt in range(TT):
                    psv = psP.tile([P, 512], F32, name=f"psv{tt}", tag="psp")
                    ts_ = slice(tt * P, (tt + 1) * P)
                    if tt < KT_PER_B:  # block-0 tokens: bf16
                        for kd in range(KD):
                            nc.tensor.matmul(psv, lhsT=xb16[:, kd, ts_],
                                             rhs=wv16g[:, kd, :],
                                             start=(kd == 0),
                                             stop=(kd == KD - 1))
                            yield
                    else:
                        for j in range(KD // 2):
                            nc.tensor.matmul(psv,
                                             lhsT=xbig[:, 2 * j:2 * j + 2, ts_],
                                             rhs=wv8g[:, 2 * j:2 * j + 2, :],
                                             start=(j == 0),
                                             stop=(j == KD // 2 - 1),
                                             perf_mode=DR)
                            yield
                    nc.vector.tensor_copy(out=vh8[:, tt, :], in_=psv)
                    if tt < KT_PER_B:
                        nc.vector.tensor_copy(out=vh16[:, tt, :], in_=psv)

            def qk_fill_gen(ws, tiles):
                """Q then K projection chains (dh-major out [dh, tokens]),
                yielding after every matmul."""
                wqh8, wkh8, wqh16, wkh16 = ws
                qt_, kt_ = tiles
                for w8, w16, dst in ((wqh8, wqh16, qt_), (wkh8, wkh16, kt_)):
                    for tb in range(TB):
                        ts_ = slice(tb * 512, (tb + 1) * 512)
                        ps = psP.tile([P, 512], F32, name="psfill", tag="psp")
                        if tb == 0:  # block-0 q,k: bf16
                            for kd in range(KD):
                                nc.tensor.matmul(ps, lhsT=w16[:, kd, :],
                                                 rhs=xb16[:, kd, :],
                                                 start=(kd == 0),
                                                 stop=(kd == KD - 1))
                                yield
                        else:
                            for j in range(KD // 2):
                                nc.tensor.matmul(
                                    ps, lhsT=w8[:, 2 * j:2 * j + 2, :],
                                    rhs=xbig[:, 2 * j:2 * j + 2, ts_],
                                    start=(j == 0),
                                    stop=(j == KD // 2 - 1),
                                    perf_mode=DR)
                                yield
                        nc.vector.tensor_copy(out=dst[:, ts_], in_=ps)

            def pump(gen, n):
                for _ in range(n):
                    try:
                        next(gen)
                    except StopIteration:
                        return False
                return True

            def pump_n(gen, n):
                c = 0
                for _ in range(n):
                    try:
                        next(gen)
                        c += 1
                    except StopIteration:
                        break
                return c

            def oproj_nb_gen(nb):
                """o_proj chains for one token block, yielding per matmul.
                Weight tiles prefetch two chains ahead so the drain phase
                never stalls the PE on a woh DMA."""
                wtag, wdram, wdt = (("woh16", wo16_d, BF16) if nb == 0
                                    else ("woh8", wo8_d, F8))
                wohs = {}

                def fetch(m):
                    woh = work.tile([P, H, P], wdt, name=f"{wtag}_{m}",
                                    tag=wtag, bufs=4)
                    nc.sync.dma_start(out=woh.rearrange("p h f -> p (h f)"),
                                      in_=wdram[m])
                    wohs[m] = woh

                ns = slice(nb * 512, (nb + 1) * 512)
                fetch(0)
                fetch(1)
                for m in range(MT):
                    if m + 2 < MT:
                        fetch(m + 2)
                    yield
                    woh = wohs.pop(m)
                    ps = psP.tile([P, 512], F32, name="psout", tag="psp")
                    if nb == 0:
                        for hh in range(H):
                            nc.tensor.matmul(ps, lhsT=woh[:, hh, :],
                                             rhs=ot016[:, hh, :],
                                             start=(hh == 0),
                                             stop=(hh == H - 1))
                            yield
                    else:
                        os_ = slice((nb - 1) * 512, nb * 512)
                        for i in range(H // 2):
                            nc.tensor.matmul(
                                ps, lhsT=woh[:, 2 * i:2 * i + 2, :],
                                rhs=ot8[:, 2 * i:2 * i + 2, os_],
                                start=(i == 0), stop=(i == H // 2 - 1),
                                perf_mode=DR)
                            yield
                    stg = work.tile([P, 512], F32, name="ostage", tag="ostage",
                                    bufs=4)
                    nc.vector.tensor_copy(out=stg, in_=ps)
                    nc.sync.dma_start(out=out_d[m * P:(m + 1) * P, ns],
                                      in_=stg)

            # ---- prologue: V for heads 0-3, then Q/K for head 0
            vws = [load_v_weights(0, w16=wv16g0, w8=wv8g0)] + [None] * (G - 1)
            vtiles = [alloc_v_tiles(0)] + [None] * (G - 1)
            g0 = vgroup_gen(vws[0], vtiles[0])
            while pump(g0, 1):
                pass
            ws_list = [None] * (H + 2)
            ws_list[0] = load_qk_weights(0)
            if H > 1:
                ws_list[1] = load_qk_weights(1)
            qt0 = work.tile([P, T], BF16, name="qt0", tag="qt")
            kt0 = work.tile([P, T], BF16, name="kt0", tag="kt")
            cur_qk = (qt0, kt0)
            g1 = qk_fill_gen(ws_list[0], cur_qk)
            while pump(g1, 1):
                pass

            fills = []

            def pump_fills(n):
                while n > 0 and fills:
                    n -= pump_n(fills[0], n)
                    if n > 0:
                        fills.pop(0)

            for h in range(H):
                qt_, kt_ = cur_qk
                g = h // 4
                vh8, vh16 = vtiles[g]
                hh = h % 4  # head index within the V group
                # prefetch weights two heads ahead
                if h + 2 < H:
                    ws_list[h + 2] = load_qk_weights(h + 2)
                if h + 1 < H:
                    next_qk = (
                        work.tile([P, T], BF16, name=f"qt{h + 1}", tag="qt"),
                        work.tile([P, T], BF16, name=f"kt{h + 1}", tag="kt"))
                    fills.append(qk_fill_gen(ws_list[h + 1], next_qk))
                else:
                    next_qk = None
                if h == 1 and G > 1:
                    vws[1] = load_v_weights(1)
                if h == 2 and G > 1:
                    vtiles[1] = alloc_v_tiles(1)
                    fills.append(vgroup_gen(vws[1], vtiles[1]))

                # causal attention, S^T layout (keys on partitions).
                # qb0: all-bf16 P/V path. qb>=1: fp8; full key tiles pair
                # into DoubleRow AV matmuls, diagonal tiles are narrowed
                # singles. Only the first 128 live columns of a diagonal
                # tile are triangular -> one [128,128] mask multiply.
                # Row sums accumulate in psR via all-ones stationary MMs.
                vsl = slice(hh * P, (hh + 1) * P)
                for qb in range(TB):
                    nkt = KT_PER_B * (qb + 1)
                    psO_t = psO.tile([P, 512], F32, name="psodt", tag="pso")
                    psR_t = psR.tile([P, 512], F32, name="psrow", tag="psr")
                    ptp = None
                    for kt in range(nkt):
                        di = kt - KT_PER_B * qb
                        c0 = max(di, 0) * P  # first live query column
                        w = 512 - c0
                        qs = slice(qb * 512 + c0, (qb + 1) * 512)
                        psS_t = psS.tile([P, 512], F32, name="pssc", tag="pss")
                        nc.tensor.matmul(psS_t[:, :w],
                                         lhsT=kt_[:, kt * P:(kt + 1) * P],
                                         rhs=qt_[:, qs],
                                         start=True, stop=True)
                        if qb == 0:
                            pt = work.tile([P, 512], BF16, name="pt16",
                                           tag="pt16", bufs=4)
                            pts = pt[:, :w]
                        elif di < 0:  # paired full tiles
                            if kt % 2 == 0:
                                ptp = work.tile([P, 2, 512], F8, name="ptp8",
                                                tag="ptp8", bufs=3)
                            pts = ptp[:, kt % 2, :]
                        else:  # diagonal singles
                            pt = work.tile([P, 512], F8, name="ptd8",
                                           tag="ptd8", bufs=4)
                            pts = pt[:, :w]
                        nc.scalar.activation(
                            out=pts, in_=psS_t[:, :w],
                            func=mybir.ActivationFunctionType.Exp, scale=qsc)
                        if di >= 0:
                            nc.vector.tensor_mul(pts[:, :P], pts[:, :P],
                                                 cm16 if qb == 0 else cm8)
                        # AV + ones row-sum accumulation
                        if qb == 0:
                            nc.tensor.matmul(psO_t[:, c0:],
                                             lhsT=vh16[:, kt, vsl], rhs=pts,
                                             start=(kt == 0),
                                             stop=(kt == nkt - 1),
                                             skip_group_check=True)
                            nc.tensor.matmul(psR_t[:, c0:], lhsT=ones,
                                             rhs=pts, start=(kt == 0),
                                             stop=(kt == nkt - 1),
                                             skip_group_check=True)
                        elif di < 0:
                            if kt % 2 == 1:
                                nc.tensor.matmul(
                                    psO_t,
                                    lhsT=vh8[:, kt - 1:kt + 1, vsl],
                                    rhs=ptp, start=(kt == 1), stop=False,
                                    perf_mode=DR, skip_group_check=True)
                                nc.tensor.matmul(
                                    psR_t, lhsT=ones8, rhs=ptp,
                                    start=(kt == 1), stop=False,
                                    perf_mode=DR, skip_group_check=True)
                        else:
                            nc.tensor.matmul(psO_t[:, c0:],
                                             lhsT=vh8[:, kt, vsl], rhs=pts,
                                             start=False,
                                             stop=(kt == nkt - 1),
                                             skip_group_check=True)
                            nc.tensor.matmul(psR_t[:, c0:],
                                             lhsT=ones8[:, 0, :], rhs=pts,
                                             start=False,
                                             stop=(kt == nkt - 1),
                                             skip_group_check=True)
                        pump_fills(2 + (kt & 1))
                    rec = work.tile([P, 512], F32, name="rec", tag="rec")
                    nc.vector.reciprocal_approx_fast(out=rec, in_=psR_t)
                    if qb == 0:
                        nc.vector.tensor_mul(ot016[:, h, :], psO_t, rec)
                    else:
                        nc.vector.tensor_mul(
                            ot8[:, h, (qb - 1) * 512:qb * 512], psO_t, rec)
                    if h == H - 1:
                        # this token block's ot columns are complete for
                        # every head: its o_proj chains become fill work
                        fills.append(oproj_nb_gen(qb))
                    pump_fills(4)
                if h < H - 1:
                    # finish next head's projections before its attention
                    while fills:
                        pump_fills(64)
                cur_qk = next_qk
            # drain remaining o_proj work
            while fills:
                pump_fills(64)

    nc.compile()
    return nc


def _bit_ternary(w):
    """Ternary BitLinear weight and its gamma: w_eff = q * gamma."""
    w = np.asarray(w, dtype=np.float32)
    gamma = max(np.float32(np.abs(w).mean(dtype=np.float32)), np.float32(1e-5))
    q = np.clip(np.round(w / gamma), -1.0, 1.0).astype(np.float32)
    return q, gamma


def _causal_mask():
    k = np.arange(128)[:, None]
    q = np.arange(128)[None, :]
    return (k <= q).astype(np.float32)


def _tile_qkv(w_shard):
    """[F, D] -> [H, 128, KD*128]: [h, p, kd*128+f] = w_shard[h*128+f, kd*128+p]."""
    Fs, Ds = w_shard.shape
    a = w_shard.reshape(Fs // 128, 128, Ds // 128, 128)  # [h, f, kd, p]
    a = a.transpose(0, 3, 2, 1).reshape(Fs // 128, 128, Ds)
    return np.ascontiguousarray(a)


def _group_v(tv):
    """[H, 128, KD*128] -> [G, 128, KD*512] 4-head groups:
    [g, p, kd*512 + hh*128 + f] = tv[4g+hh, p, kd*128+f]."""
    Hn, _, Dn = tv.shape
    KDn = Dn // 128
    a = tv.reshape(Hn // 4, 4, 128, KDn, 128)  # [g, hh, p, kd, f]
    a = a.transpose(0, 2, 3, 1, 4).reshape(Hn // 4, 128, KDn * 512)
    return np.ascontiguousarray(a)


def _tile_wo(wo_shard):
    """[D, F] -> [MT, 128, H*128]: [m, p, h*128+j] = wo_shard[m*128+j, h*128+p]."""
    Ds, Fs = wo_shard.shape
    a = wo_shard.reshape(Ds // 128, 128, Fs // 128, 128)  # [m, j, h, p]
    a = a.transpose(0, 3, 2, 1).reshape(Ds // 128, 128, Fs)
    return np.ascontiguousarray(a)


def _prep_inputs(x, wq, wk, wv, wo):
    bf = ml_dtypes.bfloat16
    x = np.asarray(x, dtype=np.float32)
    tern = {}
    gam = {}
    for n, w in (("wq", wq), ("wk", wk), ("wv", wv), ("wo", wo)):
        tern[n], gam[n] = _bit_ternary(w)
    cm = _causal_mask()
    qsc = np.full((128, 1),
                  gam["wq"] * gam["wk"] / np.float32(math.sqrt(D_HEAD)),
                  np.float32)
    xT8s, xT16s = [], []
    for b in range(B):
        xt = np.ascontiguousarray(x[b].T)
        xT8s.append(xt.astype(E4))
        xT16s.append(np.ascontiguousarray(xt[:, :512]).astype(bf))
    shards = []
    for hg in range(2):
        rows = slice(hg * F_LOC, (hg + 1) * F_LOC)
        tq = _tile_qkv(tern["wq"][rows, :])
        tk = _tile_qkv(tern["wk"][rows, :])
        tv = _group_v(_tile_qkv(tern["wv"][rows, :]))
        to = _tile_wo(tern["wo"][:, rows])
        shards.append({
            "wq8": tq.astype(E4), "wk8": tk.astype(E4),
            "wq16": tq.astype(bf), "wk16": tk.astype(bf),
            "wv8": tv.astype(E4), "wv16": tv.astype(bf),
            "wo8": to.astype(E4), "wo16": to.astype(bf),
        })
    in_maps = []
    for c in range(N_CORES):
        b, hg = c // 2, c % 2
        m = {"xT8": xT8s[b], "xT16": xT16s[b], "cm8": cm.astype(E4),
             "cm16": cm.astype(bf), "qsc": qsc}
        m.update(shards[hg])
        in_maps.append(m)
    return in_maps, np.float32(gam["wv"] * gam["wo"])


_NC_CACHE = {}


def _get_nc():
    if "nc" not in _NC_CACHE:
        _NC_CACHE["nc"] = build_bass()
    return _NC_CACHE["nc"]


def run(x, wq, wk, wv, wo, trace=False):
    nc = _get_nc()
    in_maps, oscale = _prep_inputs(x, wq, wk, wv, wo)
    res = bass_utils.run_bass_kernel_spmd(
        nc, in_maps, core_ids=list(range(N_CORES)), trace=trace)
    out = np.empty((B, T_FULL, D_MODEL), dtype=np.float32)
    for b in range(B):
        out[b] = (res.results[2 * b]["outT"]
                  + res.results[2 * b + 1]["outT"]).T * oscale
    return out, res


def kernel(x, wq, wk, wv, wo):
    out, _ = run(x, wq, wk, wv, wo)
    return out


# revision 12
# speedup vs baseline: 1.2158x; 1.0004x over previous
"""BitSelfAttention on 8 TRN2 NeuronCores — fp8 DoubleRow version.

Sharding: core c handles batch b = c//2 and head-group hg = c%2 (8 of 16 heads).
Each core computes its 8 heads' QKV projections + causal attention + its slice
of the o_proj GEMM, producing a partial output (transposed, [D, T], fp32).
Host pre-quantizes BitLinear weights to TERNARY {-1,0,+1} (exact in fp8/bf16)
and folds the gammas out: gamma_q*gamma_k/sqrt(dh) rides the exp() scale
operand on device; gamma_v*gamma_o is applied on host to the final output.

Precision plan (validated by numpy simulation against the 2e-2 gate):
the output error is dominated by EARLY tokens (short causal rows average few
v's: token 0's attention output is exactly v[0]), so the first 512-token
block runs bf16 end-to-end while blocks 1-3 run fp8 e4m3 with DoubleRow
matmuls (2 fp8 contraction rows per PE cell, ~1.8x the bf16 matmul rate):
  - Q/K projections: token block 0 bf16, blocks 1-3 fp8 DoubleRow.
  - V is projected in TRANSPOSED orientation (x tile stationary, weights
    moving) producing token-major [tok, 4*dh] tiles directly — no PE
    transposes. Four heads share one chain so the moving operand is 512
    wide and the 256-column DoubleRow LDWEIGHTS stays hidden.
  - S^T matmul: bf16 always (contraction dh=128 cannot pair).
  - exp: query block 0 -> bf16 P; blocks 1-3 -> fp8 P.
  - AV: qb0 bf16 singles; qb>=1 full key tiles as fp8 DoubleRow pairs,
    diagonal tiles as fp8 normal-rate singles (narrowed to live columns).
  - o_proj: token block 0 bf16, blocks 1-3 fp8 DoubleRow over head pairs.
Softmax denominators accumulate on the PE: an all-ones stationary matmul per
key tile (DoubleRow-paired where P is paired) adds the partition-reduced
row sums into a dedicated PSUM bank, already broadcast across partitions;
normalize by fast-reciprocal multiply during PSUM eviction. Cross-head fill
interleaving as in the bf16 kernel: the next head's projection chains (and
the last head's per-block o_proj chains) are pumped between attention ops so
the in-order PE always has work while exp results are in flight.
"""

import math

import ml_dtypes
import numpy as np

import concourse.mybir as mybir
import concourse.tile as tile
from concourse import bacc
from concourse import bass_utils

BF16 = mybir.dt.bfloat16
F32 = mybir.dt.float32
F8 = mybir.dt.float8e4
E4 = ml_dtypes.float8_e4m3
DR = mybir.MatmulPerfMode.DoubleRow

D_MODEL = 2048
N_HEAD = 16
D_HEAD = 128
B = 4
T_FULL = 2048
N_CORES = 8
F_LOC = D_MODEL // 2  # features per core (8 heads)


def build_bass(T=T_FULL, D=D_MODEL, F=F_LOC, debug=False):
    """Build the single-core program (SPMD across 8 cores via input data)."""
    P = 128
    KD = D // P      # contraction 128-tiles
    TT = T // P      # token 128-tiles
    TB = T // 512    # token 512-blocks
    H = F // P       # local heads
    MT = D // P      # output-dmodel 128-tiles
    KT_PER_B = 512 // P
    G = H // 4       # 4-head V groups

    nc = bacc.Bacc("TRN2", target_bir_lowering=False, debug=debug,
                   num_devices=N_CORES)
    xT8_d = nc.dram_tensor("xT8", [D, T], F8, kind="ExternalInput").ap()
    xT16_d = nc.dram_tensor("xT16", [D, 512], BF16, kind="ExternalInput").ap()
    # ternary weights pre-tiled on host (contiguous DMAs):
    #   wq/wk: [H, 128, KD*128] with [h, p, kd*128+f] = w[h*128+f, kd*128+p]
    #   wv:    [G, 128, KD*512] with [g, p, kd*512+hh*128+f]
    #            = wv[(4g+hh)*128+f, kd*128+p]
    #   wo:    [MT, 128, H*128] with [m, p, h*128+j] = wo[m*128+j, h*128+p]
    wq8_d = nc.dram_tensor("wq8", [H, P, KD * P], F8, kind="ExternalInput").ap()
    wk8_d = nc.dram_tensor("wk8", [H, P, KD * P], F8, kind="ExternalInput").ap()
    wq16_d = nc.dram_tensor("wq16", [H, P, KD * P], BF16,
                            kind="ExternalInput").ap()
    wk16_d = nc.dram_tensor("wk16", [H, P, KD * P], BF16,
                            kind="ExternalInput").ap()
    wv8_d = nc.dram_tensor("wv8", [G, P, KD * 512], F8,
                           kind="ExternalInput").ap()
    wv16_d = nc.dram_tensor("wv16", [G, P, KD * 512], BF16,
                            kind="ExternalInput").ap()
    wo8_d = nc.dram_tensor("wo8", [MT, P, H * P], F8, kind="ExternalInput").ap()
    wo16_d = nc.dram_tensor("wo16", [MT, P, H * P], BF16,
                            kind="ExternalInput").ap()
    cm8_d = nc.dram_tensor("cm8", [P, P], F8, kind="ExternalInput").ap()
    cm16_d = nc.dram_tensor("cm16", [P, P], BF16, kind="ExternalInput").ap()
    qsc_d = nc.dram_tensor("qsc", [P, 1], F32, kind="ExternalInput").ap()
    out_d = nc.dram_tensor("outT", [D, T], F32, kind="ExternalOutput").ap()

    with tile.TileContext(nc) as tc:
        with (
            tc.tile_pool(name="big", bufs=1) as big,
            tc.tile_pool(name="work", bufs=2) as work,
            tc.tile_pool(name="psS", bufs=3, space="PSUM") as psS,
            tc.tile_pool(name="psO", bufs=2, space="PSUM") as psO,
            tc.tile_pool(name="psR", bufs=1, space="PSUM") as psR,
            tc.tile_pool(name="psP", bufs=2, space="PSUM") as psP,
        ):
            # ---- persistent inputs, ordered so the first V chains (bf16,
            # group 0) can start while the fp8 x image still streams in.
            wv16g0 = work.tile([P, KD, 512], BF16, name="wv16g0", tag="wv16g",
                               bufs=1)
            nc.sync.dma_start(out=wv16g0.rearrange("p kd f -> p (kd f)"),
                              in_=wv16_d[0])
            xb16 = big.tile([P, KD, 512], BF16, name="xb16", tag="xb16", bufs=1)
            for kd in range(KD):
                nc.sync.dma_start(out=xb16[:, kd, :],
                                  in_=xT16_d[kd * P:(kd + 1) * P, :])
            ones = big.tile([P, P], BF16, name="ones_sb", tag="ones", bufs=1)
            nc.vector.memset(ones, 1.0)
            cm8 = big.tile([P, P], F8, name="cm8_sb", tag="cm8", bufs=1)
            nc.sync.dma_start(out=cm8, in_=cm8_d)
            cm16 = big.tile([P, P], BF16, name="cm16_sb", tag="cm16", bufs=1)
            nc.sync.dma_start(out=cm16, in_=cm16_d)
            qsc = big.tile([P, 1], F32, name="qsc_sb", tag="qsc", bufs=1)
            nc.sync.dma_start(out=qsc, in_=qsc_d)
            wv8g0 = work.tile([P, KD, 512], F8, name="wv8g0", tag="wv8g",
                              bufs=1)
            nc.sync.dma_start(out=wv8g0.rearrange("p kd f -> p (kd f)"),
                              in_=wv8_d[0])
            xbig = big.tile([P, KD, T], F8, name="xbig", tag="xbig", bufs=1)
            for kd in range(KD):
                nc.sync.dma_start(out=xbig[:, kd, :],
                                  in_=xT8_d[kd * P:(kd + 1) * P, :])
            # attention outputs: block 0 bf16, blocks 1-3 fp8 (unscaled v-hat)
            ot016 = big.tile([P, H, 512], BF16, name="ot016", tag="ot016",
                             bufs=1)
            ot8 = big.tile([P, H, T - 512], F8, name="ot8", tag="ot8", bufs=1)

            def load_qk_weights(h):
                wqh8 = work.tile([P, KD, P], F8, name=f"wqh8_{h}", tag="wqh8")
                nc.sync.dma_start(out=wqh8.rearrange("p kd f -> p (kd f)"),
                                  in_=wq8_d[h])
                wkh8 = work.tile([P, KD, P], F8, name=f"wkh8_{h}", tag="wkh8")
                nc.sync.dma_start(out=wkh8.rearrange("p kd f -> p (kd f)"),
                                  in_=wk8_d[h])
                wqh16 = work.tile([P, KD, P], BF16, name=f"wqh16_{h}",
                                  tag="wqh16")
                nc.sync.dma_start(out=wqh16.rearrange("p kd f -> p (kd f)"),
                                  in_=wq16_d[h])
                wkh16 = work.tile([P, KD, P], BF16, name=f"wkh16_{h}",
                                  tag="wkh16")
                nc.sync.dma_start(out=wkh16.rearrange("p kd f -> p (kd f)"),
                                  in_=wk16_d[h])
                return wqh8, wkh8, wqh16, wkh16

            def load_v_weights(g, w16=None, w8=None):
                if w16 is None:
                    w16 = work.tile([P, KD, 512], BF16, name=f"wv16g{g}",
                                    tag="wv16g", bufs=1)
                    nc.sync.dma_start(out=w16.rearrange("p kd f -> p (kd f)"),
                                      in_=wv16_d[g])
                if w8 is None:
                    w8 = work.tile([P, KD, 512], F8, name=f"wv8g{g}",
                                   tag="wv8g", bufs=1)
                    nc.sync.dma_start(out=w8.rearrange("p kd f -> p (kd f)"),
                                      in_=wv8_d[g])
                return w16, w8

            def alloc_v_tiles(g):
                vh8 = work.tile([P, TT, 512], F8, name=f"vh8g{g}", tag="vh8g")
                vh16 = work.tile([P, KT_PER_B, 512], BF16, name=f"vh16g{g}",
                                 tag="vh16g")
                return vh8, vh16

            def vgroup_gen(vws, vtiles):
                """Transposed V projection for a 4-head group: token-major
                [tok, 4*dh] tiles, one chain per token tile."""
                wv16g, wv8g = vws
                vh8, vh16 = vtiles
                for tt in range(TT):
                    psv = psP.tile([P, 512], F32, name=f"psv{tt}", tag="psp")
                    ts_ = slice(tt * P, (tt + 1) * P)
                    if tt < KT_PER_B:  # block-0 tokens: bf16
                        for kd in range(KD):
                            nc.tensor.matmul(psv, lhsT=xb16[:, kd, ts_],
                                             rhs=wv16g[:, kd, :],
                                             start=(kd == 0),
                                             stop=(kd == KD - 1))
                            yield
                    else:
                        for j in range(KD // 2):
                            nc.tensor.matmul(psv,
                                             lhsT=xbig[:, 2 * j:2 * j + 2, ts_],
                                             rhs=wv8g[:, 2 * j:2 * j + 2, :],
                                             start=(j == 0),
                                             stop=(j == KD // 2 - 1),
                                             perf_mode=DR)
                            yield
                    nc.vector.tensor_copy(out=vh8[:, tt, :], in_=psv)
                    if tt < KT_PER_B:
                        nc.vector.tensor_copy(out=vh16[:, tt, :], in_=psv)

            def qk_fill_gen(ws, tiles):
                """Q then K projection chains (dh-major out [dh, tokens]),
                yielding after every matmul. Block 0 (bf16, fed by xb16)
                runs first for both Q and K so prologue work exists before
                the fp8 x image has landed."""
                wqh8, wkh8, wqh16, wkh16 = ws
                qt_, kt_ = tiles
                for w8, w16, dst in ((wqh8, wqh16, qt_), (wkh8, wkh16, kt_)):
                    ps = psP.tile([P, 512], F32, name="psfill", tag="psp")
                    for kd in range(KD):
                        nc.tensor.matmul(ps, lhsT=w16[:, kd, :],
                                         rhs=xb16[:, kd, :],
                                         start=(kd == 0),
                                         stop=(kd == KD - 1))
                        yield
                    nc.vector.tensor_copy(out=dst[:, 0:512], in_=ps)
                for w8, w16, dst in ((wqh8, wqh16, qt_), (wkh8, wkh16, kt_)):
                    for tb in range(1, TB):
                        ts_ = slice(tb * 512, (tb + 1) * 512)
                        ps = psP.tile([P, 512], F32, name="psfill", tag="psp")
                        for j in range(KD // 2):
                            nc.tensor.matmul(
                                ps, lhsT=w8[:, 2 * j:2 * j + 2, :],
                                rhs=xbig[:, 2 * j:2 * j + 2, ts_],
                                start=(j == 0),
                                stop=(j == KD // 2 - 1),
                                perf_mode=DR)
                            yield
                        nc.vector.tensor_copy(out=dst[:, ts_], in_=ps)

            def pump(gen, n):
                for _ in range(n):
                    try:
                        next(gen)
                    except StopIteration:
                        return False
                return True

            def pump_n(gen, n):
                c = 0
                for _ in range(n):
                    try:
                        next(gen)
                        c += 1
                    except StopIteration:
                        break
                return c

            def oproj_nb_gen(nb):
                """o_proj chains for one token block, yielding per matmul.
                Weight tiles prefetch two chains ahead so the drain phase
                never stalls the PE on a woh DMA."""
                wtag, wdram, wdt = (("woh16", wo16_d, BF16) if nb == 0
                                    else ("woh8", wo8_d, F8))
                wohs = {}

                def fetch(m):
                    woh = work.tile([P, H, P], wdt, name=f"{wtag}_{m}",
                                    tag=wtag, bufs=4)
                    nc.sync.dma_start(out=woh.rearrange("p h f -> p (h f)"),
                                      in_=wdram[m])
                    wohs[m] = woh

                ns = slice(nb * 512, (nb + 1) * 512)
                fetch(0)
                fetch(1)
                for m in range(MT):
                    if m + 2 < MT:
                        fetch(m + 2)
                    yield
                    woh = wohs.pop(m)
                    ps = psP.tile([P, 512], F32, name="psout", tag="psp")
                    if nb == 0:
                        for hh in range(H):
                            nc.tensor.matmul(ps, lhsT=woh[:, hh, :],
                                             rhs=ot016[:, hh, :],
                                             start=(hh == 0),
                                             stop=(hh == H - 1))
                            yield
                    else:
                        os_ = slice((nb - 1) * 512, nb * 512)
                        for i in range(H // 2):
                            nc.tensor.matmul(
                                ps, lhsT=woh[:, 2 * i:2 * i + 2, :],
                                rhs=ot8[:, 2 * i:2 * i + 2, os_],
                                start=(i == 0), stop=(i == H // 2 - 1),
                                perf_mode=DR)
                            yield
                    stg = work.tile([P, 512], F32, name="ostage", tag="ostage",
                                    bufs=4)
                    nc.vector.tensor_copy(out=stg, in_=ps)
                    nc.sync.dma_start(out=out_d[m * P:(m + 1) * P, ns],
                                      in_=stg)

            # ---- prologue: V for heads 0-3 and Q/K for head 0, bf16
            # (xb16-fed) chains first so the PE has work while the larger
            # fp8 x image is still streaming in.
            vws = [load_v_weights(0, w16=wv16g0, w8=wv8g0)] + [None] * (G - 1)
            vtiles = [alloc_v_tiles(0)] + [None] * (G - 1)
            ws_list = [None] * (H + 2)
            ws_list[0] = load_qk_weights(0)
            if H > 1:
                ws_list[1] = load_qk_weights(1)
            qt0 = work.tile([P, T], BF16, name="qt0", tag="qt")
            kt0 = work.tile([P, T], BF16, name="kt0", tag="kt")
            cur_qk = (qt0, kt0)
            g0 = vgroup_gen(vws[0], vtiles[0])
            g1 = qk_fill_gen(ws_list[0], cur_qk)
            pump(g0, KT_PER_B * KD)   # bf16 V chains (token tiles 0-3)
            pump(g1, 2 * KD)          # bf16 Q/K block-0 chains
            while pump(g0, 1):
                pass
            while pump(g1, 1):
                pass

            fills = []

            def pump_fills(n):
                while n > 0 and fills:
                    n -= pump_n(fills[0], n)
                    if n > 0:
                        fills.pop(0)

            for h in range(H):
                qt_, kt_ = cur_qk
                g = h // 4
                vh8, vh16 = vtiles[g]
                hh = h % 4  # head index within the V group
                # prefetch weights two heads ahead
                if h + 2 < H:
                    ws_list[h + 2] = load_qk_weights(h + 2)
                if h + 1 < H:
                    next_qk = (
                        work.tile([P, T], BF16, name=f"qt{h + 1}", tag="qt"),
                        work.tile([P, T], BF16, name=f"kt{h + 1}", tag="kt"))
                    fills.append(qk_fill_gen(ws_list[h + 1], next_qk))
                else:
                    next_qk = None
                if h == 1 and G > 1:
                    vws[1] = load_v_weights(1)
                if h == 2 and G > 1:
                    vtiles[1] = alloc_v_tiles(1)
                    fills.append(vgroup_gen(vws[1], vtiles[1]))

                # causal attention, S^T layout (keys on partitions).
                # qb0: all-bf16 P/V path. qb>=1: fp8; full key tiles pair
                # into DoubleRow AV matmuls, diagonal tiles are narrowed
                # singles. Only the first 128 live columns of a diagonal
                # tile are triangular -> one [128,128] mask multiply.
                # Row sums accumulate in psR via all-ones stationary MMs.
                vsl = slice(hh * P, (hh + 1) * P)
                for qb in range(TB):
                    nkt = KT_PER_B * (qb + 1)
                    psO_t = psO.tile([P, 512], F32, name="psodt", tag="pso")
                    racc = work.tile([P, 512], F32, name="racc", tag="racc")
                    ptp = None
                    for kt in range(nkt):
                        di = kt - KT_PER_B * qb
                        c0 = max(di, 0) * P  # first live query column
                        w = 512 - c0
                        qs = slice(qb * 512 + c0, (qb + 1) * 512)
                        psS_t = psS.tile([P, 512], F32, name="pssc", tag="pss")
                        nc.tensor.matmul(psS_t[:, :w],
                                         lhsT=kt_[:, kt * P:(kt + 1) * P],
                                         rhs=qt_[:, qs],
                                         start=True, stop=True)
                        if qb == 0:
                            pt = work.tile([P, 512], BF16, name="pt16",
                                           tag="pt16", bufs=4)
                            pts = pt[:, :w]
                        elif di < 0:  # paired full tiles
                            if kt % 2 == 0:
                                ptp = work.tile([P, 2, 512], F8, name="ptp8",
                                                tag="ptp8", bufs=3)
                            pts = ptp[:, kt % 2, :]
                        else:  # diagonal singles
                            pt = work.tile([P, 512], F8, name="ptd8",
                                           tag="ptd8", bufs=4)
                            pts = pt[:, :w]
                        nc.scalar.activation(
                            out=pts, in_=psS_t[:, :w],
                            func=mybir.ActivationFunctionType.Exp, scale=qsc)
                        if di >= 0:
                            nc.vector.tensor_mul(pts[:, :P], pts[:, :P],
                                                 cm16 if qb == 0 else cm8)
                        # fp32 running key-tile sum on DVE (hidden behind
                        # the exp pacing)
                        if kt == 0:
                            nc.vector.tensor_copy(out=racc, in_=pts)
                        else:
                            nc.vector.tensor_add(racc[:, c0:], racc[:, c0:],
                                                 pts)
                        # AV
                        if qb == 0:
                            nc.tensor.matmul(psO_t[:, c0:],
                                             lhsT=vh16[:, kt, vsl], rhs=pts,
                                             start=(kt == 0),
                                             stop=(kt == nkt - 1),
                                             skip_group_check=True)
                        elif di < 0:
                            if kt % 2 == 1:
                                nc.tensor.matmul(
                                    psO_t,
                                    lhsT=vh8[:, kt - 1:kt + 1, vsl],
                                    rhs=ptp, start=(kt == 1), stop=False,
                                    perf_mode=DR, skip_group_check=True)
                        else:
                            nc.tensor.matmul(psO_t[:, c0:],
                                             lhsT=vh8[:, kt, vsl], rhs=pts,
                                             start=False,
                                             stop=(kt == nkt - 1),
                                             skip_group_check=True)
                        pump_fills((2 + (kt & 1)) if h < H - 1 else 5)
                    raccb = work.tile([P, 512], BF16, name="raccb",
                                      tag="raccb")
                    nc.vector.tensor_copy(out=raccb, in_=racc)
                    psR_t = psR.tile([P, 512], F32, name="psrow", tag="psr")
                    nc.tensor.matmul(psR_t, lhsT=ones, rhs=raccb,
                                     start=True, stop=True)
                    rec = work.tile([P, 512], F32, name="rec", tag="rec")
                    nc.vector.reciprocal_approx_fast(out=rec, in_=psR_t)
                    if qb == 0:
                        nc.vector.tensor_mul(ot016[:, h, :], psO_t, rec)
                    else:
                        nc.vector.tensor_mul(
                            ot8[:, h, (qb - 1) * 512:qb * 512], psO_t, rec)
                    if h == H - 1:
                        # this token block's ot columns are complete for
                        # every head: its o_proj chains become fill work
                        fills.append(oproj_nb_gen(qb))
                    pump_fills(4 if h < H - 1 else 12)
                if h < H - 1:
                    # finish next head's projections before its attention
                    while fills:
                        pump_fills(64)
                cur_qk = next_qk
            # drain remaining o_proj work
            while fills:
                pump_fills(64)

    nc.compile()
    return nc


def _bit_ternary(w):
    """Ternary BitLinear weight and its gamma: w_eff = q * gamma."""
    w = np.asarray(w, dtype=np.float32)
    gamma = max(np.float32(np.abs(w).mean(dtype=np.float32)), np.float32(1e-5))
    q = np.clip(np.round(w / gamma), -1.0, 1.0).astype(np.float32)
    return q, gamma


def _causal_mask():
    k = np.arange(128)[:, None]
    q = np.arange(128)[None, :]
    return (k <= q).astype(np.float32)


def _tile_qkv(w_shard):
    """[F, D] -> [H, 128, KD*128]: [h, p, kd*128+f] = w_shard[h*128+f, kd*128+p]."""
    Fs, Ds = w_shard.shape
    a = w_shard.reshape(Fs // 128, 128, Ds // 128, 128)  # [h, f, kd, p]
    a = a.transpose(0, 3, 2, 1).reshape(Fs // 128, 128, Ds)
    return np.ascontiguousarray(a)


def _group_v(tv):
    """[H, 128, KD*128] -> [G, 128, KD*512] 4-head groups:
    [g, p, kd*512 + hh*128 + f] = tv[4g+hh, p, kd*128+f]."""
    Hn, _, Dn = tv.shape
    KDn = Dn // 128
    a = tv.reshape(Hn // 4, 4, 128, KDn, 128)  # [g, hh, p, kd, f]
    a = a.transpose(0, 2, 3, 1, 4).reshape(Hn // 4, 128, KDn * 512)
    return np.ascontiguousarray(a)


def _tile_wo(wo_shard):
    """[D, F] -> [MT, 128, H*128]: [m, p, h*128+j] = wo_shard[m*128+j, h*128+p]."""
    Ds, Fs = wo_shard.shape
    a = wo_shard.reshape(Ds // 128, 128, Fs // 128, 128)  # [m, j, h, p]
    a = a.transpose(0, 3, 2, 1).reshape(Ds // 128, 128, Fs)
    return np.ascontiguousarray(a)


def _prep_inputs(x, wq, wk, wv, wo):
    bf = ml_dtypes.bfloat16
    x = np.asarray(x, dtype=np.float32)
    tern = {}
    gam = {}
    for n, w in (("wq", wq), ("wk", wk), ("wv", wv), ("wo", wo)):
        tern[n], gam[n] = _bit_ternary(w)
    cm = _causal_mask()
    qsc = np.full((128, 1),
                  gam["wq"] * gam["wk"] / np.float32(math.sqrt(D_HEAD)),
                  np.float32)
    xT8s, xT16s = [], []
    for b in range(B):
        xt = np.ascontiguousarray(x[b].T)
        xT8s.append(xt.astype(E4))
        xT16s.append(np.ascontiguousarray(xt[:, :512]).astype(bf))
    shards = []
    for hg in range(2):
        rows = slice(hg * F_LOC, (hg + 1) * F_LOC)
        tq = _tile_qkv(tern["wq"][rows, :])
        tk = _tile_qkv(tern["wk"][rows, :])
        tv = _group_v(_tile_qkv(tern["wv"][rows, :]))
        to = _tile_wo(tern["wo"][:, rows])
        shards.append({
            "wq8": tq.astype(E4), "wk8": tk.astype(E4),
            "wq16": tq.astype(bf), "wk16": tk.astype(bf),
            "wv8": tv.astype(E4), "wv16": tv.astype(bf),
            "wo8": to.astype(E4), "wo16": to.astype(bf),
        })
    in_maps = []
    for c in range(N_CORES):
        b, hg = c // 2, c % 2
        m = {"xT8": xT8s[b], "xT16": xT16s[b], "cm8": cm.astype(E4),
             "cm16": cm.astype(bf), "qsc": qsc}
        m.update(shards[hg])
        in_maps.append(m)
    return in_maps, np.float32(gam["wv"] * gam["wo"])


_NC_CACHE = {}


def _get_nc():
    if "nc" not in _NC_CACHE:
        _NC_CACHE["nc"] = build_bass()
    return _NC_CACHE["nc"]


def run(x, wq, wk, wv, wo, trace=False):
    nc = _get_nc()
    in_maps, oscale = _prep_inputs(x, wq, wk, wv, wo)
    res = bass_utils.run_bass_kernel_spmd(
        nc, in_maps, core_ids=list(range(N_CORES)), trace=trace)
    out = np.empty((B, T_FULL, D_MODEL), dtype=np.float32)
    for b in range(B):
        out[b] = (res.results[2 * b]["outT"]
                  + res.results[2 * b + 1]["outT"]).T * oscale
    return out, res


def kernel(x, wq, wk, wv, wo):
    out, _ = run(x, wq, wk, wv, wo)
    return out
